# revision 1
# baseline (speedup 1.0000x reference)
"""AddressAwareGNN (4-layer GAT + concat pooling + MLP) on 8 Trainium2 cores.

Sharding: nodes/edges partitioned by destination-node range (graph parallel).
Per layer: local h@W in transposed (feature-major) layout, all-gather of
projected features + attention logits, per-core edge aggregation via host-built
one-hot matmuls (segment softmax stays local to the dst range), BatchNorm
batch-stats via tiny all-reduce. Pooling: sum via node->graph one-hot matmul +
all-reduce, max via per-graph row gather; classifier replicated on all cores.
"""
import os
import sys

sys.path.insert(0, "/opt/trn_rl_repo")

import heapq
import numpy as np
import ml_dtypes

import concourse.bass as bass
import concourse.mybir as mybir
import concourse.tile as tile
from concourse import bacc
from concourse.bass_utils import run_bass_kernel_spmd
from concourse.masks import make_identity

BF16 = ml_dtypes.bfloat16
F32 = mybir.dt.float32
BF = mybir.dt.bfloat16
I32 = mybir.dt.int32
AF = mybir.ActivationFunctionType
ALU = mybir.AluOpType
AX = mybir.AxisListType

N, F_IN, H, HEADS, HD, L, G, NGF, NC = 50000, 64, 256, 8, 32, 4, 64, 32, 2
EPS = 1e-5
NCORES = 8
NLOC = N // NCORES
NW = 49
WIN = 128
NLP = NW * WIN              # 6272 padded local rows
NGLOB = NLP * NCORES        # 50176
WPM = 2
NHALF = 25 * WIN               # first-AG-chunk rows (windows 0..24)
NMEGA = (NW + WPM - 1) // WPM
GPC = G // NCORES
PADG = 1024
KG = PADG // 128
DHX = H + HEADS             # 264
P = 128


# ------------------------------------------------------------------ host prep
def _prep(inputs):
    ei = np.asarray(inputs["edge_index"]).astype(np.int64)
    batch = np.asarray(inputs["batch"]).astype(np.int64)
    src = np.concatenate([ei[0], np.arange(N, dtype=np.int64)])
    dst = np.concatenate([ei[1], np.arange(N, dtype=np.int64)])
    order = np.argsort(dst, kind="stable")
    src, dst = src[order], dst[order]
    core_lo = np.searchsorted(dst, np.arange(0, N + 1, NLOC))
    deg = np.bincount(dst, minlength=N)

    plans = []
    for c in range(NCORES):
        nodes = np.arange(c * NLOC, (c + 1) * NLOC)
        d = deg[nodes]
        order_n = np.argsort(-d, kind="stable")
        cap = np.full(NW, WIN, np.int64)
        cap[NW - 1] = NLOC - (NW - 1) * WIN
        wload = np.zeros(NW, np.int64)
        wcnt = np.zeros(NW, np.int64)
        win_of = np.empty(NLOC, np.int32)
        slot_of = np.empty(NLOC, np.int32)
        heap = [(0, w) for w in range(NW)]
        heapq.heapify(heap)
        for i in order_n:
            while True:
                load, w = heapq.heappop(heap)
                if wcnt[w] < cap[w]:
                    break
            win_of[i] = w
            slot_of[i] = wcnt[w]
            wcnt[w] += 1
            wload[w] += d[i]
            if wcnt[w] < cap[w]:
                heapq.heappush(heap, (int(wload[w]), w))
        plans.append((nodes, win_of, slot_of, wload))

    SPW = int(max(int(np.ceil(p[3].max() / P)) for p in plans))
    NSUB = NW * SPW
    NSLOT = NSUB * P

    grow_of = np.full(N, -1, np.int64)
    for c, (nodes, win_of, slot_of, _) in enumerate(plans):
        grow_of[nodes] = c * NLP + win_of.astype(np.int64) * WIN + slot_of.astype(np.int64)
    global _GROW
    _GROW = grow_of

    per_core = []
    for c in range(NCORES):
        e0, e1 = core_lo[c], core_lo[c + 1]
        es, ed = src[e0:e1], dst[e0:e1]
        nodes, win_of, slot_of, _ = plans[c]
        lw = win_of[ed - c * NLOC]
        eorder = np.argsort(lw, kind="stable")
        es, ed, lw = es[eorder], ed[eorder], lw[eorder]
        wstart = np.searchsorted(lw, np.arange(NW + 1))
        SRC = np.zeros((P, NSUB), np.int32)
        SREL = np.full(NSLOT, -1, np.int32)
        for w in range(NW):
            a, b = wstart[w], wstart[w + 1]
            k = b - a
            assert k <= SPW * P, f"window overflow {k}"
            flat = w * SPW * P + np.arange(k)
            SRC[flat % P, flat // P] = grow_of[es[a:b]]
            SREL[flat] = slot_of[ed[a:b] - c * NLOC]
        pj = np.arange(NSLOT)
        pp, jj, mm = pj % P, pj // P, SREL
        real = mm >= 0
        S = np.zeros((P, NSUB, P), BF16)
        S[pp[real], jj[real], mm[real]] = 1
        ST = np.zeros((P, NSUB, P), BF16)
        ST[mm[real], jj[real], pp[real]] = 1
        ghost = np.ones((P, NW), np.float32)
        ghost[slot_of, win_of] = 0.0
        per_core.append(dict(SRC=SRC, S=np.ascontiguousarray(S.reshape(P, NSUB * P)),
                             ST=np.ascontiguousarray(ST.reshape(P, NSUB * P)), ghost=ghost))

    gs = np.searchsorted(batch, np.arange(G + 1))
    cnt = (gs[1:] - gs[:-1]).astype(np.float32)
    assert (gs[1:] - gs[:-1]).max() <= PADG
    for c in range(NCORES):
        PIX = np.zeros((P, GPC * KG), np.int32)
        for gl in range(GPC):
            g = c * GPC + gl
            nn = np.arange(gs[g], gs[g + 1])
            rows = grow_of[nn]
            rows = np.concatenate([rows, np.full(PADG - len(nn), rows[0], np.int64)])
            i = np.arange(PADG)
            PIX[i % P, gl * KG + i // P] = rows
        per_core[c]["PIX"] = PIX
        per_core[c]["cntr8"] = (1.0 / cnt[c * GPC:(c + 1) * GPC]).reshape(GPC, 1)
        gix = np.zeros((P, 1), np.int32)
        gix[0:GPC, 0] = np.arange(c * GPC, (c + 1) * GPC)
        per_core[c]["GIX"] = gix
        nodes, win_of, slot_of, _ = plans[c]
        Sg = np.zeros((P, NW, G), BF16)
        Sg[slot_of, win_of, batch[nodes]] = 1
        per_core[c]["Sg"] = np.ascontiguousarray(Sg.reshape(P, NW * G))

    def bf(x):
        return np.ascontiguousarray(np.asarray(x, np.float32)).astype(BF16)

    Wenc = bf(inputs["W_enc"])
    Wg = bf(inputs["Wg"])
    a_s = np.asarray(inputs["att_src"], np.float32)
    a_d = np.asarray(inputs["att_dst"], np.float32)
    Aattn = np.zeros((L, H, 2 * HEADS), np.float32)
    for l in range(L):
        for h in range(HEADS):
            Aattn[l, 32 * h:32 * h + 32, h] = a_s[l, h]
            Aattn[l, 32 * h:32 * h + 32, HEADS + h] = a_d[l, h]
    bnp = np.zeros((L + 1, P, 2, 2), np.float32)
    pairs = [(inputs["g_enc"], inputs["be_enc"])] + [(inputs["bn_g"][l], inputs["bn_b"][l]) for l in range(L)]
    for i, (g_, b_) in enumerate(pairs):
        g_, b_ = np.asarray(g_, np.float32), np.asarray(b_, np.float32)
        bnp[i, :, 0, 0], bnp[i, :, 1, 0] = g_[:P], g_[P:]
        bnp[i, :, 0, 1], bnp[i, :, 1, 1] = b_[:P], b_[P:]
    W1 = np.asarray(inputs["W1"], np.float32)
    W1p = np.zeros((7, P, 2 * H), np.float32)
    for kt in range(7):
        r = W1[kt * P:(kt + 1) * P]
        W1p[kt, :r.shape[0]] = r
    W2p = np.ascontiguousarray(np.asarray(inputs["W2"], np.float32)).reshape(4, P, H)
    W3p = np.ascontiguousarray(np.asarray(inputs["W3"], np.float32)).reshape(2, P, NC)
    bn1p = np.zeros((P, 4, 2), np.float32)
    bn1p[:, :, 0] = np.asarray(inputs["g1"], np.float32).reshape(4, P).T
    bn1p[:, :, 1] = np.asarray(inputs["be1"], np.float32).reshape(4, P).T
    bn2p = np.zeros((P, 2, 2), np.float32)
    bn2p[:, :, 0] = np.asarray(inputs["g2"], np.float32).reshape(2, P).T
    bn2p[:, :, 1] = np.asarray(inputs["be2"], np.float32).reshape(2, P).T
    Aattn = Aattn.astype(BF16)
    b3 = np.asarray(inputs["b3"], np.float32).reshape(NC, 1)
    gf = np.asarray(inputs["graph_features"], np.float32).reshape(G, NGF)

    x = np.asarray(inputs["x"], np.float32)
    in_maps = []
    for c in range(NCORES):
        nodes, win_of, slot_of, _ = plans[c]
        lid = win_of.astype(np.int64) * WIN + slot_of.astype(np.int64)
        xT = np.zeros((F_IN, NLP), np.float32)
        xT[:, lid] = x[nodes].T
        m = dict(per_core[c])
        m.update(xT=xT.astype(BF16), Wenc=Wenc, Wg=Wg, Aattn=Aattn, bnp=bnp,
                 W1p=W1p, W2p=W2p, W3p=W3p, bn1p=bn1p, bn2p=bn2p, b3=b3,
                 gfl=np.ascontiguousarray(gf[c * GPC:(c + 1) * GPC]))
        in_maps.append(m)
    return in_maps, SPW, NSUB


# ------------------------------------------------------------------ builder
def _build(nc, SPW, NSUB, debug=False):
    MSUB = WPM * SPW
    NCH = (NLP + 511) // 512
    RG = [list(range(NCORES))]

    d_SRC = nc.dram_tensor("SRC", [P, NSUB], I32, kind="ExternalInput")
    d_S = nc.dram_tensor("S", [P, NSUB * P], BF, kind="ExternalInput")
    d_ST = nc.dram_tensor("ST", [P, NSUB * P], BF, kind="ExternalInput")
    d_gh = nc.dram_tensor("ghost", [P, NW], F32, kind="ExternalInput")
    d_PIX = nc.dram_tensor("PIX", [P, GPC * KG], I32, kind="ExternalInput")
    d_GIX = nc.dram_tensor("GIX", [P, 1], I32, kind="ExternalInput")
    d_cnt = nc.dram_tensor("cntr8", [GPC, 1], F32, kind="ExternalInput")
    d_Sg = nc.dram_tensor("Sg", [P, NW * G], BF, kind="ExternalInput")
    d_xT = nc.dram_tensor("xT", [F_IN, NLP], BF, kind="ExternalInput")
    d_Wenc = nc.dram_tensor("Wenc", [F_IN, H], BF, kind="ExternalInput")
    d_Wg = nc.dram_tensor("Wg", [L, H, H], BF, kind="ExternalInput")
    d_At = nc.dram_tensor("Aattn", [L, H, 2 * HEADS], BF, kind="ExternalInput")
    d_bnp = nc.dram_tensor("bnp", [L + 1, P, 2, 2], F32, kind="ExternalInput")
    d_W1 = nc.dram_tensor("W1p", [7, P, 2 * H], F32, kind="ExternalInput")
    d_W2 = nc.dram_tensor("W2p", [4, P, H], F32, kind="ExternalInput")
    d_W3 = nc.dram_tensor("W3p", [2, P, NC], F32, kind="ExternalInput")
    d_bn1 = nc.dram_tensor("bn1p", [P, 4, 2], F32, kind="ExternalInput")
    d_bn2 = nc.dram_tensor("bn2p", [P, 2, 2], F32, kind="ExternalInput")
    d_b3 = nc.dram_tensor("b3", [NC, 1], F32, kind="ExternalInput")
    d_gfl = nc.dram_tensor("gfl", [GPC, NGF], F32, kind="ExternalInput")
    d_out = nc.dram_tensor("out", [G, NC], F32, kind="ExternalOutput")
    if debug:
        d_dh = [nc.dram_tensor(f"dbg_h{i}", [P, 2, NLP], BF, kind="ExternalOutput")
                for i in range(L + 1)]
        d_dp = nc.dram_tensor("dbg_pooled", [G, 3 * H + NGF], F32, kind="ExternalOutput")

    with tile.TileContext(nc, trace_sim=False) as tc:
        with (
            tc.tile_pool(name="sb", bufs=1) as sb,
            tc.tile_pool(name="psmm", bufs=2, space="PSUM") as ps_mm,
            tc.tile_pool(name="pswin", bufs=2, space="PSUM") as ps_win,
            tc.tile_pool(name="pstr", bufs=2, space="PSUM") as ps_tr,
            tc.tile_pool(name="dr", bufs=2, space="DRAM") as dr,
        ):
            idf = sb.tile([P, P], F32)
            make_identity(nc, idf[:])
            idb = sb.tile([P, P], BF)
            make_identity(nc, idb[:])
            eps_sb = sb.tile([P, 1], F32)
            nc.vector.memset(eps_sb[:], EPS)

            src_sb = sb.tile([P, NSUB], I32)
            nc.sync.dma_start(src_sb[:], d_SRC[:])
            ghost_sb = sb.tile([P, NW], F32)
            nc.sync.dma_start(ghost_sb[:], d_gh[:])

            bnp_sb = sb.tile([P, L + 1, 2, 2], F32)
            nc.sync.dma_start(bnp_sb[:], d_bnp[:].rearrange("l p b k -> p l b k"))

            hT = sb.tile([P, 2, NLP], BF)
            d_prev = dr.tile([P, 2, NLP], BF, tag="prev", bufs=1)
            hpT = sb.tile([P, 2, NLP], BF)
            asadT = sb.tile([2 * HEADS, NLP], BF)
            zT = sb.tile([P, 2, NLP], F32)
            ad_w = sb.tile([P, NW, HEADS], BF)

            def batchnorm_relu(lay, dst_tile, residual=False):
                stats = sb.tile([P, 4], F32, tag="bnstats", bufs=2)
                nc.vector.reduce_sum(stats[:, 0:1], zT[:, 0, :], axis=AX.X)
                nc.vector.reduce_sum(stats[:, 1:2], zT[:, 1, :], axis=AX.X)
                nc.scalar.activation(hpT[:, 0, :], zT[:, 0, :], AF.Square, accum_out=stats[:, 2:3])
                nc.scalar.activation(hpT[:, 1, :], zT[:, 1, :], AF.Square, accum_out=stats[:, 3:4])
                sin = dr.tile([P, 4], F32, tag="bnin")
                sout = dr.tile([P, 4], F32, tag="bnout", addr_space="Shared")
                nc.sync.dma_start(sin[:], stats[:])
                nc.gpsimd.collective_compute("AllReduce", ALU.add, replica_groups=RG,
                                             ins=[sin[:].opt()], outs=[sout[:].opt()])
                st = sb.tile([P, 4], F32, tag="bnst", bufs=2)
                nc.sync.dma_start(st[:], sout[:])
                mu = sb.tile([P, 2], F32, tag="bnmu", bufs=2)
                nc.vector.tensor_scalar_mul(mu[:], st[:, 0:2], 1.0 / N)
                var = sb.tile([P, 2], F32, tag="bnvar", bufs=2)
                nc.vector.tensor_scalar_mul(var[:], st[:, 2:4], 1.0 / N)
                musq = sb.tile([P, 2], F32, tag="bnmusq", bufs=2)
                nc.vector.tensor_mul(musq[:], mu[:], mu[:])
                nc.vector.tensor_sub(var[:], var[:], musq[:])
                rs = sb.tile([P, 2], F32, tag="bnrs", bufs=2)
                nc.scalar.activation(rs[:], var[:], AF.Sqrt, bias=eps_sb[:, 0:1])
                nc.vector.reciprocal(rs[:], rs[:])
                Sc = sb.tile([P, 2], F32, tag="bnS", bufs=2)
                nc.vector.tensor_mul(Sc[:], rs[:], bnp_sb[:, lay, :, 0])
                Bi = sb.tile([P, 2], F32, tag="bnB", bufs=2)
                nc.vector.tensor_mul(Bi[:], mu[:], Sc[:])
                nc.vector.tensor_sub(Bi[:], bnp_sb[:, lay, :, 1], Bi[:])
                for cc in range(2):
                    c0, c1 = cc * (NLP // 2), (cc + 1) * (NLP // 2)
                    for b in range(2):
                        nc.scalar.activation(dst_tile[:, b, c0:c1], zT[:, b, c0:c1], AF.Relu,
                                             bias=Bi[:, b:b + 1], scale=Sc[:, b:b + 1])


            # ---------------- encoder ----------------
            with tc.tile_pool(name="encp", bufs=1) as ep:
                xT_sb = ep.tile([F_IN, NLP], BF)
                nc.sync.dma_start(xT_sb[:], d_xT[:])
                wenc_sb = ep.tile([F_IN, H], BF)
                nc.sync.dma_start(wenc_sb[:], d_Wenc[:])
                for ch in range(NCH):
                    f0 = ch * 512
                    F = min(512, NLP - f0)
                    for kb in range(2):
                        pz = ps_mm.tile([P, 512], F32, tag="mm")
                        nc.tensor.matmul(pz[:, :F], wenc_sb[:, kb * P:(kb + 1) * P],
                                         xT_sb[:, f0:f0 + F], start=True, stop=True)
                        nc.scalar.activation(zT[:, kb, f0:f0 + F], pz[:, :F], AF.Copy)
                batchnorm_relu(0, hT)
            if debug:
                nc.sync.dma_start(d_dh[0][:], hT[:])

            # ---------------- GAT layers ----------------
            with (
                tc.tile_pool(name="edge", bufs=1) as eb,
            ):
                for l in range(L):
                    wg_sb = sb.tile([P, 2, H], BF, tag="wg", bufs=2)
                    nc.sync.dma_start(wg_sb[:], d_Wg[l].rearrange("(t p) k -> p t k", p=P))
                    at_sb = sb.tile([P, 2, 2 * HEADS], BF, tag="at", bufs=2)
                    nc.sync.dma_start(at_sb[:], d_At[l].rearrange("(t p) k -> p t k", p=P))

                    for ch in range(NCH):
                        f0 = ch * 512
                        F = min(512, NLP - f0)
                        for kb in range(2):
                            pz = ps_mm.tile([P, 512], F32, tag="mm")
                            for jt in range(2):
                                nc.tensor.matmul(pz[:, :F], wg_sb[:, jt, kb * P:(kb + 1) * P],
                                                 hT[:, jt, f0:f0 + F], start=(jt == 0), stop=(jt == 1))
                            nc.vector.tensor_copy(hpT[:, kb, f0:f0 + F], pz[:, :F])
                        pa = ps_mm.tile([2 * HEADS, 512], F32, tag="mm")
                        for jt in range(2):
                            nc.tensor.matmul(pa[:, :F], at_sb[:, jt, :], hpT[:, jt, f0:f0 + F],
                                             start=(jt == 0), stop=(jt == 1))
                        nc.vector.tensor_copy(asadT[:, f0:f0 + F], pa[:, :F])

                    hx_loc = dr.tile([NLP, DHX], BF, tag="hxloc")
                    for w in range(NW):
                        n0 = w * WIN
                        hxw = eb.tile([P, DHX], BF, tag="hxw", bufs=3)
                        for b in range(2):
                            pt = ps_tr.tile([P, P], BF, tag="trb", bufs=2)
                            nc.tensor.transpose(out=pt[:], in_=hpT[:, b, n0:n0 + P], identity=idb[:])
                            nc.vector.tensor_copy(hxw[:, b * P:(b + 1) * P], pt[:])
                        pt2 = ps_tr.tile([P, P], BF, tag="trb", bufs=2)
                        nc.tensor.transpose(out=pt2[:, 0:2 * HEADS], in_=asadT[:, n0:n0 + P],
                                            identity=idb[0:2 * HEADS, 0:2 * HEADS])
                        nc.vector.tensor_copy(hxw[:, H:H + HEADS], pt2[:, 0:HEADS])
                        nc.vector.tensor_copy(ad_w[:, w, :], pt2[:, HEADS:2 * HEADS])
                        nc.sync.dma_start(hx_loc[n0:n0 + P, :], hxw[:])

                    hx_full = dr.tile([NCORES, NLP, DHX], BF, tag="hxfull", addr_space="Shared")
                    nc.gpsimd.collective_compute("AllGather", ALU.bypass, replica_groups=RG,
                                                 ins=[hx_loc[:].opt()],
                                                 outs=[hx_full[:].opt()])

                    for m in range(NMEGA):
                        w0 = m * WPM
                        nw_m = min(WPM, NW - w0)
                        ns_m = nw_m * SPW
                        j0 = w0 * SPW
                        Gt = eb.tile([P, MSUB, DHX], BF, tag="G", bufs=3)
                        St = eb.tile([P, MSUB * P], BF, tag="S", bufs=2)
                        STt = eb.tile([P, MSUB * P], BF, tag="ST", bufs=1)
                        nc.sync.dma_start(St[:, :ns_m * P], d_S[:, j0 * P:(j0 + ns_m) * P])
                        nc.sync.dma_start(STt[:, :ns_m * P], d_ST[:, j0 * P:(j0 + ns_m) * P])
                        et = eb.tile([P, MSUB, HEADS], F32, tag="et", bufs=2)
                        et2 = eb.tile([P, MSUB, HEADS], F32, tag="et2", bufs=1)
                        exb = eb.tile([P, MSUB, HEADS], BF, tag="exb", bufs=2)
                        for j in range(ns_m):
                            nc.gpsimd.indirect_dma_start(
                                out=Gt[:, j, :], out_offset=None,
                                in_=hx_full[:].rearrange("c n d -> (c n) d"),
                                in_offset=bass.IndirectOffsetOnAxis(
                                    ap=src_sb[:, j0 + j:j0 + j + 1], axis=0))
                            pad = ps_mm.tile([P, HEADS], F32, tag="mm")
                            nc.tensor.matmul(pad[:], STt[:, j * P:(j + 1) * P],
                                             ad_w[:, w0 + j // SPW, :], start=True, stop=True)
                            nc.vector.tensor_add(et[:, j, :], Gt[:, j, H:H + HEADS], pad[:])
                        # leaky_relu(x, 0.2) then exp
                        nc.vector.tensor_scalar_mul(et2[:, :ns_m, :], et[:, :ns_m, :], 0.2)
                        nc.vector.tensor_max(et2[:, :ns_m, :], et2[:, :ns_m, :], et[:, :ns_m, :])
                        nc.scalar.activation(exb[:, :ns_m, :], et2[:, :ns_m, :], AF.Exp)
                        rhs_t = eb.tile([P, MSUB, DHX], BF, tag="rhs", bufs=2)
                        nc.vector.tensor_mul(
                            rhs_t[:, :ns_m, 0:H].rearrange("p j (h d) -> p j h d", h=HEADS),
                            Gt[:, :ns_m, 0:H].rearrange("p j (h d) -> p j h d", h=HEADS),
                            exb[:, :ns_m, :, None].to_broadcast([P, ns_m, HEADS, HD]))
                        nc.vector.tensor_copy(rhs_t[:, :ns_m, H:H + HEADS], exb[:, :ns_m, :])
                        for wl in range(nw_m):
                            w = w0 + wl
                            pw = ps_win.tile([P, DHX], F32, tag="win")
                            for k in range(SPW):
                                j = wl * SPW + k
                                nc.tensor.matmul(pw[:], St[:, j * P:(j + 1) * P], rhs_t[:, j, :],
                                                 start=(k == 0), stop=(k == SPW - 1))
                            den = eb.tile([P, HEADS], F32, tag="den", bufs=3)
                            nc.vector.tensor_scalar(den[:], pw[:, H:H + HEADS],
                                                    scalar1=ghost_sb[:, w:w + 1],
                                                    scalar2=None, op0=ALU.add)
                            nc.vector.reciprocal(den[:], den[:])
                            zw = eb.tile([P, H], F32, tag="zw", bufs=3)
                            nc.vector.tensor_mul(
                                zw[:].rearrange("p (h d) -> p h d", h=HEADS),
                                pw[:, 0:H].rearrange("p (h d) -> p h d", h=HEADS),
                                den[:, :, None].to_broadcast([P, HEADS, HD]))
                            for b in range(2):
                                pt = ps_tr.tile([P, P], F32, tag="trf")
                                nc.tensor.transpose(out=pt[:], in_=zw[:, b * P:(b + 1) * P],
                                                    identity=idf[:])
                                nc.scalar.activation(zT[:, b, w * WIN:w * WIN + P], pt[:], AF.Copy)

                    batchnorm_relu(l + 1, hT)
                    if l == 1:
                        nc.sync.dma_start(d_prev[:], hT[:])
                    if l == 2:
                        for b in range(2):
                            for cc in range(7):
                                c0 = cc * 896
                                cw = min(896, NLP - c0)
                                psc = eb.tile([P, 896], BF, tag="prevc", bufs=2)
                                nc.sync.dma_start(psc[:, :cw], d_prev[:, b, c0:c0 + cw])
                                nc.vector.tensor_add(hT[:, b, c0:c0 + cw], hT[:, b, c0:c0 + cw],
                                                     psc[:, :cw])
                    if debug:
                        nc.sync.dma_start(d_dh[l + 1][:], hT[:])

            # ---------------- pooling ----------------
            with tc.tile_pool(name="poolp", bufs=1) as pb:
                sg_sb = pb.tile([P, NW * G], BF)
                nc.sync.dma_start(sg_sb[:], d_Sg[:])
                hf_loc = dr.tile([NLP, H], BF, tag="hfloc")
                pp0 = ps_mm.tile([G, H], F32, tag="mm")
                for w in range(NW):
                    n0 = w * WIN
                    hxw = pb.tile([P, H], BF, tag="hxw2", bufs=3)
                    for b in range(2):
                        pt = ps_tr.tile([P, P], BF, tag="trb", bufs=2)
                        nc.tensor.transpose(out=pt[:], in_=hT[:, b, n0:n0 + P], identity=idb[:])
                        nc.vector.tensor_copy(hxw[:, b * P:(b + 1) * P], pt[:])
                    nc.sync.dma_start(hf_loc[n0:n0 + P, :], hxw[:])
                    nc.tensor.matmul(pp0[:], sg_sb[:, w * G:(w + 1) * G], hxw[:],
                                     start=(w == 0), stop=(w == NW - 1))
                sum_sb = pb.tile([G, H], F32)
                nc.vector.tensor_copy(sum_sb[:], pp0[:])
                sin2 = dr.tile([G, H], F32, tag="poolin")
                sout2 = dr.tile([G, H], F32, tag="poolout", addr_space="Shared")
                nc.sync.dma_start(sin2[:], sum_sb[:])
                nc.gpsimd.collective_compute("AllReduce", ALU.add, replica_groups=RG,
                                             ins=[sin2[:].opt()], outs=[sout2[:].opt()])

                hf_full = dr.tile([NGLOB, H], BF, tag="hffull", addr_space="Shared")
                nc.gpsimd.collective_compute("AllGather", ALU.bypass, replica_groups=RG,
                                             ins=[hf_loc[:].opt()], outs=[hf_full[:].opt()])

                pix_sb = pb.tile([P, GPC * KG], I32)
                nc.sync.dma_start(pix_sb[:], d_PIX[:])
                gix_sb = pb.tile([P, 1], I32)
                nc.sync.dma_start(gix_sb[:], d_GIX[:])
                maxTg = pb.tile([P, 2, GPC], BF)
                for gl in range(GPC):
                    gg = pb.tile([P, KG, H], BF, tag="gg", bufs=2)
                    for k in range(KG):
                        nc.gpsimd.indirect_dma_start(
                            out=gg[:, k, :], out_offset=None, in_=hf_full[:],
                            in_offset=bass.IndirectOffsetOnAxis(
                                ap=pix_sb[:, gl * KG + k:gl * KG + k + 1], axis=0))
                    gt = pb.tile([P, 2, PADG], F32, tag="gt", bufs=2)
                    for k in range(KG):
                        for b in range(2):
                            pt = ps_tr.tile([P, P], BF, tag="trb", bufs=2)
                            nc.tensor.transpose(out=pt[:], in_=gg[:, k, b * P:(b + 1) * P],
                                                identity=idb[:])
                            nc.scalar.activation(gt[:, b, k * P:(k + 1) * P], pt[:], AF.Copy)
                    for b in range(2):
                        nc.vector.reduce_max(maxTg[:, b, gl:gl + 1], gt[:, b, :], axis=AX.X)

                # pooled_loc [GPC, 800] = [mean | max | sum | gf]
                pooled = pb.tile([GPC, 3 * H + NGF], F32)
                mysum = pb.tile([P, H], F32)
                nc.gpsimd.indirect_dma_start(
                    out=mysum[:], out_offset=None, in_=sout2[:],
                    in_offset=bass.IndirectOffsetOnAxis(ap=gix_sb[:, 0:1], axis=0))
                cnt_sb = pb.tile([GPC, 1], F32)
                nc.sync.dma_start(cnt_sb[:], d_cnt[:])
                nc.vector.tensor_scalar(pooled[:, 0:H], mysum[0:GPC, :], scalar1=cnt_sb[:],
                                        scalar2=None, op0=ALU.mult)
                nc.vector.tensor_copy(pooled[:, 2 * H:3 * H], mysum[0:GPC, :])
                for b in range(2):
                    pt = ps_tr.tile([P, P], BF, tag="trb", bufs=2)
                    nc.tensor.transpose(out=pt[0:GPC, 0:P], in_=maxTg[:, b, :], identity=idb[:])
                    nc.scalar.activation(pooled[:, H + b * P:H + (b + 1) * P], pt[0:GPC, 0:P],
                                         AF.Copy)
                nc.sync.dma_start(pooled[:, 3 * H:], d_gfl[:])

                pin = dr.tile([GPC, 3 * H + NGF], F32, tag="pin")
                pfull_d = dr.tile([G, 3 * H + NGF], F32, tag="pfull", addr_space="Shared")
                nc.sync.dma_start(pin[:], pooled[:])
                nc.gpsimd.collective_compute("AllGather", ALU.bypass, replica_groups=RG,
                                             ins=[pin[:].opt()], outs=[pfull_d[:].opt()])
                pfull = pb.tile([G, 3 * H + NGF], F32)
                nc.sync.dma_start(pfull[:], pfull_d[:])
                if debug:
                    nc.sync.dma_start(d_dp[:], pfull[:])

                # ---------------- classifier (replicated) ----------------
                pT = pb.tile([P, 7, G], F32)
                nc.gpsimd.memset(pT[:], 0)
                for t in range(7):
                    w_ = min(P, 3 * H + NGF - t * P)
                    pt = ps_mm.tile([P, P], F32, tag="mm")
                    nc.tensor.transpose(out=pt[0:w_, 0:G], in_=pfull[:, t * P:t * P + w_],
                                        identity=idf[0:G, 0:G])
                    nc.scalar.activation(pT[:w_, t, :], pt[:w_, 0:G], AF.Copy)

                w1_sb = pb.tile([P, 7, 2 * H], F32)
                nc.sync.dma_start(w1_sb[:], d_W1[:].rearrange("t p k -> p t k"))
                bn1_sb = pb.tile([P, 4, 2], F32)
                nc.sync.dma_start(bn1_sb[:], d_bn1[:])
                z1 = pb.tile([P, 4, G], F32)

                def mlp_bn(zt, nblk, bnsb, ngraph=G):
                    # BN over free axis (graphs) + relu, in place
                    for b in range(nblk):
                        s_ = pb.tile([P, 1], F32, tag="cbs", bufs=2)
                        nc.vector.reduce_sum(s_[:], zt[:, b, :], axis=AX.X)
                        sqt = pb.tile([P, G], F32, tag="cbsq", bufs=2)
                        q_ = pb.tile([P, 1], F32, tag="cbq", bufs=2)
                        nc.scalar.activation(sqt[:], zt[:, b, :], AF.Square, accum_out=q_[:])
                        mu = pb.tile([P, 1], F32, tag="cbmu", bufs=2)
                        nc.vector.tensor_scalar_mul(mu[:], s_[:], 1.0 / ngraph)
                        var = pb.tile([P, 1], F32, tag="cbvar", bufs=2)
                        nc.vector.tensor_scalar_mul(var[:], q_[:], 1.0 / ngraph)
                        ms = pb.tile([P, 1], F32, tag="cbms", bufs=2)
                        nc.vector.tensor_mul(ms[:], mu[:], mu[:])
                        nc.vector.tensor_sub(var[:], var[:], ms[:])
                        rs = pb.tile([P, 1], F32, tag="cbrs", bufs=2)
                        nc.scalar.activation(rs[:], var[:], AF.Sqrt, bias=eps_sb[:, 0:1])
                        nc.vector.reciprocal(rs[:], rs[:])
                        Sc = pb.tile([P, 1], F32, tag="cbS", bufs=2)
                        nc.vector.tensor_mul(Sc[:], rs[:], bnsb[:, b, 0:1])
                        Bi = pb.tile([P, 1], F32, tag="cbB", bufs=2)
                        nc.vector.tensor_mul(Bi[:], mu[:], Sc[:])
                        nc.vector.tensor_sub(Bi[:], bnsb[:, b, 1:2], Bi[:])
                        nc.scalar.activation(zt[:, b, :], zt[:, b, :], AF.Relu,
                                             bias=Bi[:], scale=Sc[:])

                for mb in range(4):
                    pz = ps_mm.tile([P, 512], F32, tag="mm")
                    for kt in range(7):
                        nc.tensor.matmul(pz[:, 0:G], w1_sb[:, kt, mb * P:(mb + 1) * P],
                                         pT[:, kt, :], start=(kt == 0), stop=(kt == 6))
                    nc.scalar.activation(z1[:, mb, :], pz[:, 0:G], AF.Copy)
                mlp_bn(z1, 4, bn1_sb)

                w2_sb = pb.tile([P, 4, H], F32)
                nc.sync.dma_start(w2_sb[:], d_W2[:].rearrange("t p k -> p t k"))
                bn2_sb = pb.tile([P, 2, 2], F32)
                nc.sync.dma_start(bn2_sb[:], d_bn2[:])
                z2 = pb.tile([P, 2, G], F32)
                for mb in range(2):
                    pz = ps_mm.tile([P, 512], F32, tag="mm")
                    for kt in range(4):
                        nc.tensor.matmul(pz[:, 0:G], w2_sb[:, kt, mb * P:(mb + 1) * P],
                                         z1[:, kt, :], start=(kt == 0), stop=(kt == 3))
                    nc.scalar.activation(z2[:, mb, :], pz[:, 0:G], AF.Copy)
                mlp_bn(z2, 2, bn2_sb)

                w3_sb = pb.tile([P, 2, NC], F32)
                nc.sync.dma_start(w3_sb[:], d_W3[:].rearrange("t p k -> p t k"))
                b3_sb = pb.tile([NC, 1], F32)
                nc.sync.dma_start(b3_sb[:], d_b3[:])
                pz3 = ps_mm.tile([P, 512], F32, tag="mm")
                for kt in range(2):
                    nc.tensor.matmul(pz3[0:NC, 0:G], w3_sb[:, kt, :], z2[:, kt, :],
                                     start=(kt == 0), stop=(kt == 1))
                z3 = pb.tile([NC, G], F32)
                nc.scalar.activation(z3[:], pz3[0:NC, 0:G], AF.Identity, bias=b3_sb[:, 0:1])
                nc.sync.dma_start(d_out[:].rearrange("g c -> c g"), z3[:])
    return nc


_CACHE = {}
_GROW = None


def _get_compiled(SPW, NSUB, debug):
    key = (SPW, NSUB, debug)
    if key not in _CACHE:
        nc = bacc.Bacc("TRN2", target_bir_lowering=False, debug=False, num_devices=NCORES)
        _build(nc, SPW, NSUB, debug=debug)
        nc.compile()
        _CACHE[key] = nc
    return _CACHE[key]


def kernel(debug=False, _want_results=False, **inputs):
    in_maps, SPW, NSUB = _prep(inputs)
    nc = _get_compiled(SPW, NSUB, debug)
    res = run_bass_kernel_spmd(nc, in_maps, core_ids=list(range(NCORES)))
    out = np.asarray(res.results[0]["out"], np.float32)
    if _want_results:
        return out, res
    return out



# revision 16
# speedup vs baseline: 1.2475x; 1.2475x over previous
"""AddressAwareGNN (4-layer GAT + concat pooling + MLP) on 8 Trainium2 cores.

Sharding: nodes/edges partitioned by destination-node range (graph parallel).
Per layer: local h@W in transposed (feature-major) layout, all-gather of
projected features + attention logits, per-core edge aggregation via host-built
one-hot matmuls (segment softmax stays local to the dst range), BatchNorm
batch-stats via tiny all-reduce. Pooling: sum via node->graph one-hot matmul +
all-reduce, max via per-graph row gather; classifier replicated on all cores.
"""
import os
import sys

sys.path.insert(0, "/opt/trn_rl_repo")

import heapq
import numpy as np
import ml_dtypes

import concourse.bass as bass
import concourse.mybir as mybir
import concourse.tile as tile
from concourse import bacc
from concourse.bass_utils import run_bass_kernel_spmd
from concourse.masks import make_identity

BF16 = ml_dtypes.bfloat16
F32 = mybir.dt.float32
BF = mybir.dt.bfloat16
I32 = mybir.dt.int32
AF = mybir.ActivationFunctionType
ALU = mybir.AluOpType
AX = mybir.AxisListType

N, F_IN, H, HEADS, HD, L, G, NGF, NC = 50000, 64, 256, 8, 32, 4, 64, 32, 2
EPS = 1e-5
NCORES = 8
NLOC = N // NCORES
NW = 49
WIN = 128
NLP = NW * WIN              # 6272 padded local rows
NGLOB = NLP * NCORES        # 50176
WPM = 2
NHALF = 25 * WIN               # first-AG-chunk rows (windows 0..24)
NMEGA = (NW + WPM - 1) // WPM
GPC = G // NCORES
PADG = 1024
KG = PADG // 128
DHX = H + HEADS             # 264
P = 128


# ------------------------------------------------------------------ host prep
def _prep(inputs):
    ei = np.asarray(inputs["edge_index"]).astype(np.int64)
    batch = np.asarray(inputs["batch"]).astype(np.int64)
    src = np.concatenate([ei[0], np.arange(N, dtype=np.int64)])
    dst = np.concatenate([ei[1], np.arange(N, dtype=np.int64)])
    order = np.argsort(dst, kind="stable")
    src, dst = src[order], dst[order]
    core_lo = np.searchsorted(dst, np.arange(0, N + 1, NLOC))
    deg = np.bincount(dst, minlength=N)

    plans = []
    for c in range(NCORES):
        nodes = np.arange(c * NLOC, (c + 1) * NLOC)
        d = deg[nodes]
        order_n = np.argsort(-d, kind="stable")
        cap = np.full(NW, WIN, np.int64)
        cap[NW - 1] = NLOC - (NW - 1) * WIN
        wload = np.zeros(NW, np.int64)
        wcnt = np.zeros(NW, np.int64)
        win_of = np.empty(NLOC, np.int32)
        slot_of = np.empty(NLOC, np.int32)
        heap = [(0, w) for w in range(NW)]
        heapq.heapify(heap)
        for i in order_n:
            while True:
                load, w = heapq.heappop(heap)
                if wcnt[w] < cap[w]:
                    break
            win_of[i] = w
            slot_of[i] = wcnt[w]
            wcnt[w] += 1
            wload[w] += d[i]
            if wcnt[w] < cap[w]:
                heapq.heappush(heap, (int(wload[w]), w))
        plans.append((nodes, win_of, slot_of, wload))

    SPW = int(max(int(np.ceil(p[3].max() / P)) for p in plans))
    NSUB = NW * SPW
    NSLOT = NSUB * P

    grow_of = np.full(N, -1, np.int64)
    for c, (nodes, win_of, slot_of, _) in enumerate(plans):
        grow_of[nodes] = c * NLP + win_of.astype(np.int64) * WIN + slot_of.astype(np.int64)
    global _GROW
    _GROW = grow_of

    per_core = []
    for c in range(NCORES):
        e0, e1 = core_lo[c], core_lo[c + 1]
        es, ed = src[e0:e1], dst[e0:e1]
        nodes, win_of, slot_of, _ = plans[c]
        lw = win_of[ed - c * NLOC]
        eorder = np.argsort(lw, kind="stable")
        es, ed, lw = es[eorder], ed[eorder], lw[eorder]
        wstart = np.searchsorted(lw, np.arange(NW + 1))
        SRC = np.zeros((P, NSUB), np.int32)
        SREL = np.full(NSLOT, -1, np.int32)
        for w in range(NW):
            a, b = wstart[w], wstart[w + 1]
            k = b - a
            assert k <= SPW * P, f"window overflow {k}"
            flat = w * SPW * P + np.arange(k)
            SRC[flat % P, flat // P] = grow_of[es[a:b]]
            SREL[flat] = slot_of[ed[a:b] - c * NLOC]
        pj = np.arange(NSLOT)
        pp, jj, mm = pj % P, pj // P, SREL
        real = mm >= 0
        S = np.zeros((P, NSUB, P), BF16)
        S[pp[real], jj[real], mm[real]] = 1
        ST = np.zeros((P, NSUB, P), BF16)
        ST[mm[real], jj[real], pp[real]] = 1
        ghost = np.ones((P, NW), np.float32)
        ghost[slot_of, win_of] = 0.0
        per_core.append(dict(SRC=SRC, S=np.ascontiguousarray(S.reshape(P, NSUB * P)),
                             ST=np.ascontiguousarray(ST.reshape(P, NSUB * P)), ghost=ghost))

    gs = np.searchsorted(batch, np.arange(G + 1))
    cnt = (gs[1:] - gs[:-1]).astype(np.float32)
    assert (gs[1:] - gs[:-1]).max() <= PADG
    for c in range(NCORES):
        PIX = np.zeros((P, GPC * KG), np.int32)
        for gl in range(GPC):
            g = c * GPC + gl
            nn = np.arange(gs[g], gs[g + 1])
            rows = grow_of[nn]
            rows = np.concatenate([rows, np.full(PADG - len(nn), rows[0], np.int64)])
            i = np.arange(PADG)
            PIX[i % P, gl * KG + i // P] = rows
        per_core[c]["PIX"] = PIX
        per_core[c]["cntr8"] = (1.0 / cnt[c * GPC:(c + 1) * GPC]).reshape(GPC, 1)
        gix = np.zeros((P, 1), np.int32)
        gix[0:GPC, 0] = np.arange(c * GPC, (c + 1) * GPC)
        per_core[c]["GIX"] = gix
        nodes, win_of, slot_of, _ = plans[c]
        Sg = np.zeros((P, NW, G), BF16)
        Sg[slot_of, win_of, batch[nodes]] = 1
        per_core[c]["Sg"] = np.ascontiguousarray(Sg.reshape(P, NW * G))

    def bf(x):
        return np.ascontiguousarray(np.asarray(x, np.float32)).astype(BF16)

    Wenc = bf(inputs["W_enc"])
    Wg = bf(inputs["Wg"])
    a_s = np.asarray(inputs["att_src"], np.float32)
    a_d = np.asarray(inputs["att_dst"], np.float32)
    Aattn = np.zeros((L, H, 2 * HEADS), np.float32)
    for l in range(L):
        for h in range(HEADS):
            Aattn[l, 32 * h:32 * h + 32, h] = a_s[l, h]
            Aattn[l, 32 * h:32 * h + 32, HEADS + h] = a_d[l, h]
    bnp = np.zeros((L + 1, P, 2, 2), np.float32)
    pairs = [(inputs["g_enc"], inputs["be_enc"])] + [(inputs["bn_g"][l], inputs["bn_b"][l]) for l in range(L)]
    for i, (g_, b_) in enumerate(pairs):
        g_, b_ = np.asarray(g_, np.float32), np.asarray(b_, np.float32)
        bnp[i, :, 0, 0], bnp[i, :, 1, 0] = g_[:P], g_[P:]
        bnp[i, :, 0, 1], bnp[i, :, 1, 1] = b_[:P], b_[P:]
    W1 = np.asarray(inputs["W1"], np.float32)
    W1p = np.zeros((7, P, 2 * H), np.float32)
    for kt in range(7):
        r = W1[kt * P:(kt + 1) * P]
        W1p[kt, :r.shape[0]] = r
    W2p = np.ascontiguousarray(np.asarray(inputs["W2"], np.float32)).reshape(4, P, H)
    W3p = np.ascontiguousarray(np.asarray(inputs["W3"], np.float32)).reshape(2, P, NC)
    bn1p = np.zeros((P, 4, 2), np.float32)
    bn1p[:, :, 0] = np.asarray(inputs["g1"], np.float32).reshape(4, P).T
    bn1p[:, :, 1] = np.asarray(inputs["be1"], np.float32).reshape(4, P).T
    bn2p = np.zeros((P, 2, 2), np.float32)
    bn2p[:, :, 0] = np.asarray(inputs["g2"], np.float32).reshape(2, P).T
    bn2p[:, :, 1] = np.asarray(inputs["be2"], np.float32).reshape(2, P).T
    Aattn = Aattn.astype(BF16)
    b3 = np.asarray(inputs["b3"], np.float32).reshape(NC, 1)
    gf = np.asarray(inputs["graph_features"], np.float32).reshape(G, NGF)

    x = np.asarray(inputs["x"], np.float32)
    in_maps = []
    for c in range(NCORES):
        nodes, win_of, slot_of, _ = plans[c]
        lid = win_of.astype(np.int64) * WIN + slot_of.astype(np.int64)
        xT = np.zeros((F_IN, NLP), np.float32)
        xT[:, lid] = x[nodes].T
        m = dict(per_core[c])
        m.update(xT=xT.astype(BF16), Wenc=Wenc, Wg=Wg, Aattn=Aattn, bnp=bnp,
                 W1p=W1p, W2p=W2p, W3p=W3p, bn1p=bn1p, bn2p=bn2p, b3=b3,
                 gfl=np.ascontiguousarray(gf[c * GPC:(c + 1) * GPC]))
        in_maps.append(m)
    return in_maps, SPW, NSUB


# ------------------------------------------------------------------ builder
def _build(nc, SPW, NSUB, debug=False):
    MSUB = WPM * SPW
    NCH = (NLP + 511) // 512
    RG = [list(range(NCORES))]

    d_SRC = nc.dram_tensor("SRC", [P, NSUB], I32, kind="ExternalInput")
    d_S = nc.dram_tensor("S", [P, NSUB * P], BF, kind="ExternalInput")
    d_ST = nc.dram_tensor("ST", [P, NSUB * P], BF, kind="ExternalInput")
    d_gh = nc.dram_tensor("ghost", [P, NW], F32, kind="ExternalInput")
    d_PIX = nc.dram_tensor("PIX", [P, GPC * KG], I32, kind="ExternalInput")
    d_GIX = nc.dram_tensor("GIX", [P, 1], I32, kind="ExternalInput")
    d_cnt = nc.dram_tensor("cntr8", [GPC, 1], F32, kind="ExternalInput")
    d_Sg = nc.dram_tensor("Sg", [P, NW * G], BF, kind="ExternalInput")
    d_xT = nc.dram_tensor("xT", [F_IN, NLP], BF, kind="ExternalInput")
    d_Wenc = nc.dram_tensor("Wenc", [F_IN, H], BF, kind="ExternalInput")
    d_Wg = nc.dram_tensor("Wg", [L, H, H], BF, kind="ExternalInput")
    d_At = nc.dram_tensor("Aattn", [L, H, 2 * HEADS], BF, kind="ExternalInput")
    d_bnp = nc.dram_tensor("bnp", [L + 1, P, 2, 2], F32, kind="ExternalInput")
    d_W1 = nc.dram_tensor("W1p", [7, P, 2 * H], F32, kind="ExternalInput")
    d_W2 = nc.dram_tensor("W2p", [4, P, H], F32, kind="ExternalInput")
    d_W3 = nc.dram_tensor("W3p", [2, P, NC], F32, kind="ExternalInput")
    d_bn1 = nc.dram_tensor("bn1p", [P, 4, 2], F32, kind="ExternalInput")
    d_bn2 = nc.dram_tensor("bn2p", [P, 2, 2], F32, kind="ExternalInput")
    d_b3 = nc.dram_tensor("b3", [NC, 1], F32, kind="ExternalInput")
    d_gfl = nc.dram_tensor("gfl", [GPC, NGF], F32, kind="ExternalInput")
    d_out = nc.dram_tensor("out", [G, NC], F32, kind="ExternalOutput")
    if debug:
        d_dh = [nc.dram_tensor(f"dbg_h{i}", [P, 2, NLP], BF, kind="ExternalOutput")
                for i in range(L + 1)]
        d_dp = nc.dram_tensor("dbg_pooled", [G, 3 * H + NGF], F32, kind="ExternalOutput")

    with tile.TileContext(nc, trace_sim=False) as tc:
        with (
            tc.tile_pool(name="sb", bufs=1) as sb,
            tc.tile_pool(name="psmm", bufs=2, space="PSUM") as ps_mm,
            tc.tile_pool(name="pswin", bufs=2, space="PSUM") as ps_win,
            tc.tile_pool(name="pstr", bufs=2, space="PSUM") as ps_tr,
            tc.tile_pool(name="dr", bufs=2, space="DRAM") as dr,
        ):
            idf = sb.tile([P, P], F32)
            make_identity(nc, idf[:])
            idb = sb.tile([P, P], BF)
            make_identity(nc, idb[:])
            eps_sb = sb.tile([P, 1], F32)
            nc.vector.memset(eps_sb[:], EPS)

            src_sb = sb.tile([P, NSUB], I32)
            nc.sync.dma_start(src_sb[:], d_SRC[:])
            ghost_sb = sb.tile([P, NW], F32)
            nc.sync.dma_start(ghost_sb[:], d_gh[:])

            bnp_sb = sb.tile([P, L + 1, 2, 2], F32)
            nc.sync.dma_start(bnp_sb[:], d_bnp[:].rearrange("l p b k -> p l b k"))

            hT = sb.tile([P, 2, NLP], BF)
            d_prev = dr.tile([P, 2, NLP], BF, tag="prev", bufs=1)
            hpT = sb.tile([P, 2, NLP], BF)
            asadT = sb.tile([2 * HEADS, NLP], BF)
            zT = sb.tile([P, 2, NLP], F32)
            ad_w = sb.tile([P, NW, HEADS], BF)

            def batchnorm_relu(lay, dst_tile, residual=False):
                stats = sb.tile([P, 4], F32, tag="bnstats", bufs=2)
                nc.vector.reduce_sum(stats[:, 0:1], zT[:, 0, :], axis=AX.X)
                nc.vector.reduce_sum(stats[:, 1:2], zT[:, 1, :], axis=AX.X)
                nc.scalar.activation(hpT[:, 0, :], zT[:, 0, :], AF.Square, accum_out=stats[:, 2:3])
                nc.scalar.activation(hpT[:, 1, :], zT[:, 1, :], AF.Square, accum_out=stats[:, 3:4])
                sin = dr.tile([P, 4], F32, tag="bnin")
                sout = dr.tile([P, 4], F32, tag="bnout", addr_space="Shared")
                nc.sync.dma_start(sin[:], stats[:])
                nc.gpsimd.collective_compute("AllReduce", ALU.add, replica_groups=RG,
                                             ins=[sin[:].opt()], outs=[sout[:].opt()])
                st = sb.tile([P, 4], F32, tag="bnst", bufs=2)
                nc.sync.dma_start(st[:], sout[:])
                mu = sb.tile([P, 2], F32, tag="bnmu", bufs=2)
                nc.vector.tensor_scalar_mul(mu[:], st[:, 0:2], 1.0 / N)
                var = sb.tile([P, 2], F32, tag="bnvar", bufs=2)
                nc.vector.tensor_scalar_mul(var[:], st[:, 2:4], 1.0 / N)
                musq = sb.tile([P, 2], F32, tag="bnmusq", bufs=2)
                nc.vector.tensor_mul(musq[:], mu[:], mu[:])
                nc.vector.tensor_sub(var[:], var[:], musq[:])
                rs = sb.tile([P, 2], F32, tag="bnrs", bufs=2)
                nc.scalar.activation(rs[:], var[:], AF.Sqrt, bias=eps_sb[:, 0:1])
                nc.vector.reciprocal(rs[:], rs[:])
                Sc = sb.tile([P, 2], F32, tag="bnS", bufs=2)
                nc.vector.tensor_mul(Sc[:], rs[:], bnp_sb[:, lay, :, 0])
                Bi = sb.tile([P, 2], F32, tag="bnB", bufs=2)
                nc.vector.tensor_mul(Bi[:], mu[:], Sc[:])
                nc.vector.tensor_sub(Bi[:], bnp_sb[:, lay, :, 1], Bi[:])
                for cc in range(2):
                    c0, c1 = cc * (NLP // 2), (cc + 1) * (NLP // 2)
                    for b in range(2):
                        nc.scalar.activation(dst_tile[:, b, c0:c1], zT[:, b, c0:c1], AF.Relu,
                                             bias=Bi[:, b:b + 1], scale=Sc[:, b:b + 1])


            # ---------------- encoder ----------------
            with tc.tile_pool(name="encp", bufs=1) as ep:
                xT_sb = ep.tile([F_IN, NLP], BF)
                nc.sync.dma_start(xT_sb[:], d_xT[:])
                wenc_sb = ep.tile([F_IN, H], BF)
                nc.sync.dma_start(wenc_sb[:], d_Wenc[:])
                for ch in range(NCH):
                    f0 = ch * 512
                    F = min(512, NLP - f0)
                    for kb in range(2):
                        pz = ps_mm.tile([P, 512], F32, tag="mm")
                        nc.tensor.matmul(pz[:, :F], wenc_sb[:, kb * P:(kb + 1) * P],
                                         xT_sb[:, f0:f0 + F], start=True, stop=True)
                        nc.scalar.activation(zT[:, kb, f0:f0 + F], pz[:, :F], AF.Copy)
                batchnorm_relu(0, hT)
            if debug:
                nc.sync.dma_start(d_dh[0][:], hT[:])

            # ---------------- GAT layers ----------------
            with (
                tc.tile_pool(name="edge", bufs=1) as eb,
            ):
                for l in range(L):
                    wg_sb = sb.tile([P, 2, H], BF, tag="wg", bufs=2)
                    nc.sync.dma_start(wg_sb[:], d_Wg[l].rearrange("(t p) k -> p t k", p=P))
                    at_sb = sb.tile([P, 2, 2 * HEADS], BF, tag="at", bufs=2)
                    nc.sync.dma_start(at_sb[:], d_At[l].rearrange("(t p) k -> p t k", p=P))

                    for ch in range(NCH):
                        f0 = ch * 512
                        F = min(512, NLP - f0)
                        for kb in range(2):
                            pz = ps_mm.tile([P, 512], F32, tag="mm")
                            for jt in range(2):
                                nc.tensor.matmul(pz[:, :F], wg_sb[:, jt, kb * P:(kb + 1) * P],
                                                 hT[:, jt, f0:f0 + F], start=(jt == 0), stop=(jt == 1))
                            nc.vector.tensor_copy(hpT[:, kb, f0:f0 + F], pz[:, :F])
                        pa = ps_mm.tile([2 * HEADS, 512], F32, tag="mm")
                        for jt in range(2):
                            nc.tensor.matmul(pa[:, :F], at_sb[:, jt, :], hpT[:, jt, f0:f0 + F],
                                             start=(jt == 0), stop=(jt == 1))
                        nc.vector.tensor_copy(asadT[:, f0:f0 + F], pa[:, :F])

                    hx_loc = dr.tile([NLP, DHX], BF, tag="hxloc")
                    for w in range(NW):
                        n0 = w * WIN
                        hxw = eb.tile([P, DHX], BF, tag="hxw", bufs=3)
                        for b in range(2):
                            pt = ps_tr.tile([P, P], BF, tag="trb", bufs=2)
                            nc.tensor.transpose(out=pt[:], in_=hpT[:, b, n0:n0 + P], identity=idb[:])
                            nc.vector.tensor_copy(hxw[:, b * P:(b + 1) * P], pt[:])
                        pt2 = ps_tr.tile([P, P], BF, tag="trb", bufs=2)
                        nc.tensor.transpose(out=pt2[:, 0:2 * HEADS], in_=asadT[:, n0:n0 + P],
                                            identity=idb[0:2 * HEADS, 0:2 * HEADS])
                        nc.vector.tensor_copy(hxw[:, H:H + HEADS], pt2[:, 0:HEADS])
                        nc.vector.tensor_copy(ad_w[:, w, :], pt2[:, HEADS:2 * HEADS])
                        nc.sync.dma_start(hx_loc[n0:n0 + P, :], hxw[:])

                    hx_full = dr.tile([NCORES, NLP, DHX], BF, tag="hxfull", addr_space="Shared")
                    nc.gpsimd.collective_compute("AllGather", ALU.bypass, replica_groups=RG,
                                                 ins=[hx_loc[:].opt()],
                                                 outs=[hx_full[:].opt()])

                    for m in range(NMEGA):
                        w0 = m * WPM
                        nw_m = min(WPM, NW - w0)
                        ns_m = nw_m * SPW
                        j0 = w0 * SPW
                        Gt = eb.tile([P, MSUB, DHX], BF, tag="G", bufs=3)
                        St = eb.tile([P, MSUB * P], BF, tag="S", bufs=2)
                        STt = eb.tile([P, MSUB * P], BF, tag="ST", bufs=1)
                        nc.sync.dma_start(St[:, :ns_m * P], d_S[:, j0 * P:(j0 + ns_m) * P])
                        nc.sync.dma_start(STt[:, :ns_m * P], d_ST[:, j0 * P:(j0 + ns_m) * P])
                        et = eb.tile([P, MSUB, HEADS], F32, tag="et", bufs=2)
                        et2 = eb.tile([P, MSUB, HEADS], F32, tag="et2", bufs=1)
                        exb = eb.tile([P, MSUB, HEADS], BF, tag="exb", bufs=2)
                        for j in range(ns_m):
                            nc.gpsimd.indirect_dma_start(
                                out=Gt[:, j, :], out_offset=None,
                                in_=hx_full[:].rearrange("c n d -> (c n) d"),
                                in_offset=bass.IndirectOffsetOnAxis(
                                    ap=src_sb[:, j0 + j:j0 + j + 1], axis=0))
                            pad = ps_mm.tile([P, HEADS], F32, tag="mm")
                            nc.tensor.matmul(pad[:], STt[:, j * P:(j + 1) * P],
                                             ad_w[:, w0 + j // SPW, :], start=True, stop=True)
                            nc.vector.tensor_add(et[:, j, :], Gt[:, j, H:H + HEADS], pad[:])
                        # leaky_relu(x, 0.2) then exp
                        nc.vector.tensor_scalar_mul(et2[:, :ns_m, :], et[:, :ns_m, :], 0.2)
                        nc.vector.tensor_max(et2[:, :ns_m, :], et2[:, :ns_m, :], et[:, :ns_m, :])
                        nc.scalar.activation(exb[:, :ns_m, :], et2[:, :ns_m, :], AF.Exp)
                        rhs_t = eb.tile([P, MSUB, DHX], BF, tag="rhs", bufs=2)
                        nc.vector.tensor_mul(
                            rhs_t[:, :ns_m, 0:H].rearrange("p j (h d) -> p j h d", h=HEADS),
                            Gt[:, :ns_m, 0:H].rearrange("p j (h d) -> p j h d", h=HEADS),
                            exb[:, :ns_m, :, None].to_broadcast([P, ns_m, HEADS, HD]))
                        nc.vector.tensor_copy(rhs_t[:, :ns_m, H:H + HEADS], exb[:, :ns_m, :])
                        for wl in range(nw_m):
                            w = w0 + wl
                            pw = ps_win.tile([P, DHX], F32, tag="win")
                            for k in range(SPW):
                                j = wl * SPW + k
                                nc.tensor.matmul(pw[:], St[:, j * P:(j + 1) * P], rhs_t[:, j, :],
                                                 start=(k == 0), stop=(k == SPW - 1))
                            den = eb.tile([P, HEADS], F32, tag="den", bufs=3)
                            nc.vector.tensor_scalar(den[:], pw[:, H:H + HEADS],
                                                    scalar1=ghost_sb[:, w:w + 1],
                                                    scalar2=None, op0=ALU.add)
                            nc.vector.reciprocal(den[:], den[:])
                            zw = eb.tile([P, H], F32, tag="zw", bufs=3)
                            nc.vector.tensor_mul(
                                zw[:].rearrange("p (h d) -> p h d", h=HEADS),
                                pw[:, 0:H].rearrange("p (h d) -> p h d", h=HEADS),
                                den[:, :, None].to_broadcast([P, HEADS, HD]))
                            for b in range(2):
                                pt = ps_tr.tile([P, P], F32, tag="trf")
                                nc.tensor.transpose(out=pt[:], in_=zw[:, b * P:(b + 1) * P],
                                                    identity=idf[:])
                                nc.scalar.activation(zT[:, b, w * WIN:w * WIN + P], pt[:], AF.Copy)

                    batchnorm_relu(l + 1, hT)
                    if l == 1:
                        nc.sync.dma_start(d_prev[:], hT[:])
                    if l == 2:
                        for b in range(2):
                            for cc in range(7):
                                c0 = cc * 896
                                cw = min(896, NLP - c0)
                                psc = eb.tile([P, 896], BF, tag="prevc", bufs=2)
                                nc.sync.dma_start(psc[:, :cw], d_prev[:, b, c0:c0 + cw])
                                nc.vector.tensor_add(hT[:, b, c0:c0 + cw], hT[:, b, c0:c0 + cw],
                                                     psc[:, :cw])
                    if debug:
                        nc.sync.dma_start(d_dh[l + 1][:], hT[:])

            # ---------------- pooling ----------------
            with tc.tile_pool(name="poolp", bufs=1) as pb:
                sg_sb = pb.tile([P, NW * G], BF)
                nc.sync.dma_start(sg_sb[:], d_Sg[:])
                hf_loc = dr.tile([NLP, H], BF, tag="hfloc")
                pp0 = ps_mm.tile([G, H], F32, tag="mm")
                for w in range(NW):
                    n0 = w * WIN
                    hxw = pb.tile([P, H], BF, tag="hxw2", bufs=3)
                    for b in range(2):
                        pt = ps_tr.tile([P, P], BF, tag="trb", bufs=2)
                        nc.tensor.transpose(out=pt[:], in_=hT[:, b, n0:n0 + P], identity=idb[:])
                        nc.vector.tensor_copy(hxw[:, b * P:(b + 1) * P], pt[:])
                    nc.sync.dma_start(hf_loc[n0:n0 + P, :], hxw[:])
                    nc.tensor.matmul(pp0[:], sg_sb[:, w * G:(w + 1) * G], hxw[:],
                                     start=(w == 0), stop=(w == NW - 1))
                sum_sb = pb.tile([G, H], F32)
                nc.vector.tensor_copy(sum_sb[:], pp0[:])
                sin2 = dr.tile([G, H], F32, tag="poolin")
                sout2 = dr.tile([G, H], F32, tag="poolout", addr_space="Shared")
                nc.sync.dma_start(sin2[:], sum_sb[:])
                nc.gpsimd.collective_compute("AllReduce", ALU.add, replica_groups=RG,
                                             ins=[sin2[:].opt()], outs=[sout2[:].opt()])

                hf_full = dr.tile([NGLOB, H], BF, tag="hffull", addr_space="Shared")
                nc.gpsimd.collective_compute("AllGather", ALU.bypass, replica_groups=RG,
                                             ins=[hf_loc[:].opt()], outs=[hf_full[:].opt()])

                pix_sb = pb.tile([P, GPC * KG], I32)
                nc.sync.dma_start(pix_sb[:], d_PIX[:])
                gix_sb = pb.tile([P, 1], I32)
                nc.sync.dma_start(gix_sb[:], d_GIX[:])
                maxTg = pb.tile([P, 2, GPC], BF)
                for gl in range(GPC):
                    gg = pb.tile([P, KG, H], BF, tag="gg", bufs=2)
                    for k in range(KG):
                        nc.gpsimd.indirect_dma_start(
                            out=gg[:, k, :], out_offset=None, in_=hf_full[:],
                            in_offset=bass.IndirectOffsetOnAxis(
                                ap=pix_sb[:, gl * KG + k:gl * KG + k + 1], axis=0))
                    gt = pb.tile([P, 2, PADG], F32, tag="gt", bufs=2)
                    for k in range(KG):
                        for b in range(2):
                            pt = ps_tr.tile([P, P], BF, tag="trb", bufs=2)
                            nc.tensor.transpose(out=pt[:], in_=gg[:, k, b * P:(b + 1) * P],
                                                identity=idb[:])
                            nc.scalar.activation(gt[:, b, k * P:(k + 1) * P], pt[:], AF.Copy)
                    for b in range(2):
                        nc.vector.reduce_max(maxTg[:, b, gl:gl + 1], gt[:, b, :], axis=AX.X)

                # pooled_loc [GPC, 800] = [mean | max | sum | gf]
                pooled = pb.tile([GPC, 3 * H + NGF], F32)
                mysum = pb.tile([P, H], F32)
                nc.gpsimd.indirect_dma_start(
                    out=mysum[:], out_offset=None, in_=sout2[:],
                    in_offset=bass.IndirectOffsetOnAxis(ap=gix_sb[:, 0:1], axis=0))
                cnt_sb = pb.tile([GPC, 1], F32)
                nc.sync.dma_start(cnt_sb[:], d_cnt[:])
                nc.vector.tensor_scalar(pooled[:, 0:H], mysum[0:GPC, :], scalar1=cnt_sb[:],
                                        scalar2=None, op0=ALU.mult)
                nc.vector.tensor_copy(pooled[:, 2 * H:3 * H], mysum[0:GPC, :])
                for b in range(2):
                    pt = ps_tr.tile([P, P], BF, tag="trb", bufs=2)
                    nc.tensor.transpose(out=pt[0:GPC, 0:P], in_=maxTg[:, b, :], identity=idb[:])
                    nc.scalar.activation(pooled[:, H + b * P:H + (b + 1) * P], pt[0:GPC, 0:P],
                                         AF.Copy)
                nc.sync.dma_start(pooled[:, 3 * H:], d_gfl[:])

                pin = dr.tile([GPC, 3 * H + NGF], F32, tag="pin")
                pfull_d = dr.tile([G, 3 * H + NGF], F32, tag="pfull", addr_space="Shared")
                nc.sync.dma_start(pin[:], pooled[:])
                nc.gpsimd.collective_compute("AllGather", ALU.bypass, replica_groups=RG,
                                             ins=[pin[:].opt()], outs=[pfull_d[:].opt()])
                pfull = pb.tile([G, 3 * H + NGF], F32)
                nc.sync.dma_start(pfull[:], pfull_d[:])
                if debug:
                    nc.sync.dma_start(d_dp[:], pfull[:])

                # ---------------- classifier (replicated) ----------------
                pT = pb.tile([P, 7, G], F32)
                nc.gpsimd.memset(pT[:], 0)
                for t in range(7):
                    w_ = min(P, 3 * H + NGF - t * P)
                    pt = ps_mm.tile([P, P], F32, tag="mm")
                    nc.tensor.transpose(out=pt[0:w_, 0:G], in_=pfull[:, t * P:t * P + w_],
                                        identity=idf[0:G, 0:G])
                    nc.scalar.activation(pT[:w_, t, :], pt[:w_, 0:G], AF.Copy)

                w1_sb = pb.tile([P, 7, 2 * H], F32)
                nc.sync.dma_start(w1_sb[:], d_W1[:].rearrange("t p k -> p t k"))
                bn1_sb = pb.tile([P, 4, 2], F32)
                nc.sync.dma_start(bn1_sb[:], d_bn1[:])
                z1 = pb.tile([P, 4, G], F32)

                def mlp_bn(zt, nblk, bnsb, ngraph=G):
                    # BN over free axis (graphs) + relu, in place
                    for b in range(nblk):
                        s_ = pb.tile([P, 1], F32, tag="cbs", bufs=2)
                        nc.vector.reduce_sum(s_[:], zt[:, b, :], axis=AX.X)
                        sqt = pb.tile([P, G], F32, tag="cbsq", bufs=2)
                        q_ = pb.tile([P, 1], F32, tag="cbq", bufs=2)
                        nc.scalar.activation(sqt[:], zt[:, b, :], AF.Square, accum_out=q_[:])
                        mu = pb.tile([P, 1], F32, tag="cbmu", bufs=2)
                        nc.vector.tensor_scalar_mul(mu[:], s_[:], 1.0 / ngraph)
                        var = pb.tile([P, 1], F32, tag="cbvar", bufs=2)
                        nc.vector.tensor_scalar_mul(var[:], q_[:], 1.0 / ngraph)
                        ms = pb.tile([P, 1], F32, tag="cbms", bufs=2)
                        nc.vector.tensor_mul(ms[:], mu[:], mu[:])
                        nc.vector.tensor_sub(var[:], var[:], ms[:])
                        rs = pb.tile([P, 1], F32, tag="cbrs", bufs=2)
                        nc.scalar.activation(rs[:], var[:], AF.Sqrt, bias=eps_sb[:, 0:1])
                        nc.vector.reciprocal(rs[:], rs[:])
                        Sc = pb.tile([P, 1], F32, tag="cbS", bufs=2)
                        nc.vector.tensor_mul(Sc[:], rs[:], bnsb[:, b, 0:1])
                        Bi = pb.tile([P, 1], F32, tag="cbB", bufs=2)
                        nc.vector.tensor_mul(Bi[:], mu[:], Sc[:])
                        nc.vector.tensor_sub(Bi[:], bnsb[:, b, 1:2], Bi[:])
                        nc.scalar.activation(zt[:, b, :], zt[:, b, :], AF.Relu,
                                             bias=Bi[:], scale=Sc[:])

                for mb in range(4):
                    pz = ps_mm.tile([P, 512], F32, tag="mm")
                    for kt in range(7):
                        nc.tensor.matmul(pz[:, 0:G], w1_sb[:, kt, mb * P:(mb + 1) * P],
                                         pT[:, kt, :], start=(kt == 0), stop=(kt == 6))
                    nc.scalar.activation(z1[:, mb, :], pz[:, 0:G], AF.Copy)
                mlp_bn(z1, 4, bn1_sb)

                w2_sb = pb.tile([P, 4, H], F32)
                nc.sync.dma_start(w2_sb[:], d_W2[:].rearrange("t p k -> p t k"))
                bn2_sb = pb.tile([P, 2, 2], F32)
                nc.sync.dma_start(bn2_sb[:], d_bn2[:])
                z2 = pb.tile([P, 2, G], F32)
                for mb in range(2):
                    pz = ps_mm.tile([P, 512], F32, tag="mm")
                    for kt in range(4):
                        nc.tensor.matmul(pz[:, 0:G], w2_sb[:, kt, mb * P:(mb + 1) * P],
                                         z1[:, kt, :], start=(kt == 0), stop=(kt == 3))
                    nc.scalar.activation(z2[:, mb, :], pz[:, 0:G], AF.Copy)
                mlp_bn(z2, 2, bn2_sb)

                w3_sb = pb.tile([P, 2, NC], F32)
                nc.sync.dma_start(w3_sb[:], d_W3[:].rearrange("t p k -> p t k"))
                b3_sb = pb.tile([NC, 1], F32)
                nc.sync.dma_start(b3_sb[:], d_b3[:])
                pz3 = ps_mm.tile([P, 512], F32, tag="mm")
                for kt in range(2):
                    nc.tensor.matmul(pz3[0:NC, 0:G], w3_sb[:, kt, :], z2[:, kt, :],
                                     start=(kt == 0), stop=(kt == 1))
                z3 = pb.tile([NC, G], F32)
                nc.scalar.activation(z3[:], pz3[0:NC, 0:G], AF.Identity, bias=b3_sb[:, 0:1])
                nc.sync.dma_start(d_out[:].rearrange("g c -> c g"), z3[:])
    return nc


_CACHE = {}
_GROW = None


def _get_compiled(SPW, NSUB, debug):
    key = (SPW, NSUB, debug)
    if key not in _CACHE:
        nc = bacc.Bacc("TRN2", target_bir_lowering=False, debug=False, num_devices=NCORES)
        _build(nc, SPW, NSUB, debug=debug)
        nc.compile()
        _CACHE[key] = nc
    return _CACHE[key]


def kernel(debug=False, _want_results=False, **inputs):
    in_maps, SPW, NSUB = _prep(inputs)
    nc = _get_compiled(SPW, NSUB, debug)
    res = run_bass_kernel_spmd(nc, in_maps, core_ids=list(range(NCORES)))
    out = np.asarray(res.results[0]["out"], np.float32)
    if _want_results:
        return out, res
    return out

